# revision 9
# baseline (speedup 1.0000x reference)
"""Channel-wise min/max stats kernel for Trainium2 (8 NeuronCores).

Input:  tensor [1024, 32768] float32
Output: (min_vals [1024], max_vals [1024]) float32  -- per-channel min/max

Sharding: channel axis split across 8 cores (128 channels each -> exactly the
128 SBUF partitions). Each core reduces its own rows; host concatenates.
No collectives needed.

Per-core kernel (raw Bass, manual sems), a 3-stage chunk pipeline:

  DMA (ACT ring)  -> f32 chunk lands in SBUF
  ACT             -> casts the chunk f32->bf16 into a 2-slot ring buffer
                     (an in-place bitcast cast simulates correctly but
                     corrupts on real HW -- the engine's read prefetch and
                     write streams are not strictly element-ordered)
  DVE             -> one tensor_scalar per stat (min/max): op0 is an identity
                     (min(x,+BIG) / max(x,-BIG)), op1+accum_out reduce the
                     chunk into per-chunk partials in ONE instruction.  With
                     2-byte operands the DVE runs its 2x/4x perf mode
                     (f32 measured 1.06 ns/elem; bf16 targets 0.26-0.53).
                     DVE incs a consumed-sem per chunk; ACT waits on it
                     before reusing a ring slot.

This takes DVE off the critical path and leaves the input DMA as the
bottleneck (~43 us: 16 MiB/core at the ~390 GB/s per-core HBM-read roofline,
16 SDMA engines measured 95% busy).  bf16 rounding bounds the result error
at ~2^-9 relative, far inside the 2e-2 harness gate.  (tensor_tensor_reduce
would reduce f32 at 2 elem/cycle but hits a walrus codegen bug "ISA wrong
length"; tensor_tensor_scan compiles but runs at 2 cycles/elem; f32
tensor_scalar accum measured 1x despite the cost model's 2x_2p claim.)
Chunks are ordered big->small so the post-last-byte tail (ACT cast + 2
tensor_scalars on a 256-col chunk + 2 [P,N_CHUNKS] partial reduces + the
[128,2] store) stays ~2.5 us.
"""

import sys
from contextlib import ExitStack

for _p in ("/opt/trn_rl_repo",):
    if _p not in sys.path:
        sys.path.insert(0, _p)

import numpy as np

import concourse.bass as bass
import concourse.mybir as mybir
from concourse.bass_utils import run_bass_kernel_spmd

P = 128            # partitions = channels per core
W = 32768          # elements per channel
C = 1024           # total channels
N_CORES = 8
# Big chunks first for DMA efficiency, descending tail so the final chunk's
# pipeline drain (the only part serialized after the last DMA byte) is tiny.
CHUNKS = [8192, 8192, 8192, 4096, 2048, 1024, 512, 256, 256]
assert sum(CHUNKS) == W
N_CHUNKS = len(CHUNKS)
OFFS = [sum(CHUNKS[:j]) for j in range(N_CHUNKS)]
SINK_W = max(CHUNKS)
RING_SLOTS = 2     # bf16 cast ring: ACT fills slot j%2, DVE drains it

FLT_BIG = 3.0e38   # identity bound for min(x, +BIG) / max(x, -BIG)

_NC_CACHE = {}


def _build_bass(detect_races=False):
    f32 = mybir.dt.float32
    bf16 = mybir.dt.bfloat16
    # Bass.__init__ unconditionally emits 4 GpSimd memsets for const APs
    # (0.0/1.0/bf16-1.0/u8-127) that this kernel never reads; they delay
    # GpSimd's arrival at the entry barrier. Skip their emission.
    _orig_memset = bass.BassGpSimd.memset
    bass.BassGpSimd.memset = lambda self, ap, constant: None
    try:
        nc = bass.Bass(detect_race_conditions=detect_races)
    finally:
        bass.BassGpSimd.memset = _orig_memset
    x = nc.declare_dram_parameter("x", [P, W], f32, isOutput=False)
    mnmx_out = nc.declare_dram_parameter("mnmx", [P, 2], f32, isOutput=True)

    with ExitStack() as ctx:
        data = ctx.enter_context(nc.sbuf_tensor("data", [P, W], f32))
        ring = ctx.enter_context(
            nc.sbuf_tensor("ring", [P, RING_SLOTS * SINK_W], bf16)
        )
        sink = ctx.enter_context(nc.sbuf_tensor("sink", [P, SINK_W], bf16))
        pmin = ctx.enter_context(nc.sbuf_tensor("pmin", [P, N_CHUNKS], f32))
        pmax = ctx.enter_context(nc.sbuf_tensor("pmax", [P, N_CHUNKS], f32))
        mnmx = ctx.enter_context(nc.sbuf_tensor("mnmx_sb", [P, 2], f32))
        ld_sem = ctx.enter_context(nc.semaphore("ld"))
        cv_sem = ctx.enter_context(nc.semaphore("cv"))
        rc_sem = ctx.enter_context(nc.semaphore("rc"))
        sem_v = ctx.enter_context(nc.semaphore("vec_done"))
        sem_st = ctx.enter_context(nc.semaphore("st_done"))
        block = ctx.enter_context(nc.Block())

        def bf_view(j):
            base = (j % RING_SLOTS) * SINK_W
            return ring[:, base : base + CHUNKS[j]]

        # Loads issued from the ACT HWDGE ring: its engine preamble retires
        # slightly before the SP ring's, so the first chunk lands earlier.
        # All issues go out before the convert loop so the DMA queue never
        # starves behind a conversion.
        @block.scalar
        def _(scalar):
            for j in range(N_CHUNKS):
                sl = slice(OFFS[j], OFFS[j] + CHUNKS[j])
                scalar.dma_start(out=data[:, sl], in_=x[:, sl]).then_inc(
                    ld_sem, 16
                )
            for j in range(N_CHUNKS):
                off, S = OFFS[j], CHUNKS[j]
                scalar.wait_ge(ld_sem, 16 * (j + 1))
                if j >= RING_SLOTS:
                    scalar.wait_ge(rc_sem, j - RING_SLOTS + 1)
                scalar.activation(
                    out=bf_view(j),
                    in_=data[:, off : off + S],
                    func=mybir.ActivationFunctionType.Copy,
                ).then_inc(cv_sem, 1)

        @block.sync
        def _(sync):
            sync.wait_ge(sem_v, 1)
            sync.dma_start(out=mnmx_out[:], in_=mnmx[:]).then_inc(sem_st, 16)
            # production relies on the block-exit DGE drain to quiesce the
            # output DMA before NEFF completion

        @block.vector
        def _(vector):
            for j in range(N_CHUNKS):
                S = CHUNKS[j]
                vector.wait_ge(cv_sem, j + 1)
                for dst, op, bound in ((pmin, mybir.AluOpType.min, FLT_BIG),
                                       (pmax, mybir.AluOpType.max, -FLT_BIG)):
                    ins = nc.vector.tensor_scalar(
                        out=sink[:, :S],
                        in0=bf_view(j),
                        scalar1=bound,
                        scalar2=None,
                        op0=op,
                        op1=op,
                        accum_out=dst[:, j : j + 1],
                    )
                ins.then_inc(rc_sem, 1)
            nc.vector.tensor_reduce(
                out=mnmx[:, 0:1], in_=pmin[:], axis=mybir.AxisListType.X,
                op=mybir.AluOpType.min,
            )
            nc.vector.tensor_reduce(
                out=mnmx[:, 1:2], in_=pmax[:], axis=mybir.AxisListType.X,
                op=mybir.AluOpType.max,
            ).then_inc(sem_v, 1)

    return nc


def _get_nc():
    if "nc" not in _NC_CACHE:
        _NC_CACHE["nc"] = _build_bass()
    return _NC_CACHE["nc"]


def run(tensor, trace=False):
    """Run the SPMD kernel; returns (min_vals, max_vals, BassKernelResults)."""
    x = np.ascontiguousarray(np.asarray(tensor, dtype=np.float32))
    assert x.shape == (C, W), x.shape
    in_maps = [
        {"x": np.ascontiguousarray(x[i * P : (i + 1) * P])} for i in range(N_CORES)
    ]
    nc = _get_nc()
    out = run_bass_kernel_spmd(nc, in_maps, core_ids=list(range(N_CORES)), trace=trace)
    mins = np.concatenate([r["mnmx"][:, 0] for r in out.results])
    maxs = np.concatenate([r["mnmx"][:, 1] for r in out.results])
    return mins, maxs, out


def kernel(tensor):
    mins, maxs, _ = run(tensor, trace=False)
    return mins, maxs
